# revision 42
# baseline (speedup 1.0000x reference)
"""Trainium2 Bass kernel for nn_DNFLayer (fuzzy DNF layer).

Strategy
--------
Data-parallel over batch B=32 across 8 cores (4 batches/core). Per core the
(i, j) permutation grid is padded to the full 32x32 grid (diagonal masked via
the OR-kernel broadcast), giving 4096 rows = 32 row-tiles of 128 partitions.

The conjunct product over the 112 inputs is factorized per permutation
(i, j):  conj = F0(b) * FU1(b,i) * FU2(b,j) * FB1(b,i,j) * FB2(b,j,i),
each factor a product of per-channel affine terms (alpha*x + beta), with
(alpha, beta) derived on-device from softmax(and_kernel); the per-(r,d) beta
products fold into the OR-kernel broadcast via Ln / matmul column-sum / Exp.

The per-permutation binary work (32 channels x 24 (r,d) x 1024 perms x 4
batches) is split across engines to balance the NeuronCore:

* Batches 0,1 (DVE path): "h-form" evaluation
  prod(gamma x + 1) = prod(clamped gamma) * prod(x + 1/gamma), i.e. one
  bf16 tensor_tensor ADD (2x DVE mode) + a pairwise product tree, with the
  gamma-product G8 folded in at the product-of-8 stage to keep bf16
  magnitudes bounded (gamma is clamped to |gamma| >= 2e-4 first; the shift
  changes each factor by <= 2e-4 which is far inside the accuracy budget).

* Batches 2,3 (ACT/PE log path): the Scalar engine computes
  ln(gamma*x + 1) directly from a channel-major replica of x (gamma as a
  per-partition scale vector, bias=1), the PE segment-sums the 32 channels
  per (r,d) with 0/1 selector matmuls accumulated in PSUM, ACT exponentiates
  the PSUM, and the PE transposes conj[rd, perm] back to perm-major via
  identity matmuls. ln values are fp16 (exact enough: abs err ~2^-11 per
  term) so all matmuls are single-pass.

Both paths produce the same cj[perm, (k, rd)] tile; a batched tail applies
PFOK (unary/nullary factors x sigmoid(or_kernel) x diagonal mask), the
probabilistic-sum trees over disjuncts, and fp32 merges with the residual
inputs. The ACT table sequence is kept to Exp -> Ln -> Exp (3 loads) by
deferring bA = exp(sum ln beta) until after all Ln work.
"""

import numpy as np
import ml_dtypes

BF = ml_dtypes.bfloat16
B, N, P0, P1, P2, R, D = 32, 32, 16, 32, 16, 3, 8
RD = R * D              # 24
NCORE = 8
BL = B // NCORE         # 4 batches per core
NT = BL * 8             # 32 row-tiles of 128 per core

_CACHE = {}


def _build():
    import concourse.tile as tile
    from concourse import mybir, bacc

    from concourse.masks import make_identity as _make_identity

    F32 = mybir.dt.float32
    B16 = mybir.dt.bfloat16
    MUL = mybir.AluOpType.mult
    ADD = mybir.AluOpType.add
    SUB = mybir.AluOpType.subtract
    GE = mybir.AluOpType.is_ge
    AX = mybir.AxisListType.X
    AF = mybir.ActivationFunctionType
    EPS = 2e-4

    nc = bacc.Bacc("TRN2", target_bir_lowering=False, debug=False,
                   num_devices=NCORE)

    # ---- parameters (per-core shards / replicated constants) ----
    x_all_in = nc.declare_dram_parameter("x_all", [128, NT * 32], B16, isOutput=False)
    xr_in = nc.declare_dram_parameter("xr", [128, 2048], B16, isOutput=False)
    xu_in = nc.declare_dram_parameter("xu", [128, 80], B16, isOutput=False)
    akt_in = nc.declare_dram_parameter("akt", [112, 72], F32, isOutput=False)
    ork_in = nc.declare_dram_parameter("ork", [1, 24], F32, isOutput=False)
    sel_in = nc.declare_dram_parameter("selcat", [32, 1152], B16, isOutput=False)
    mask_in = nc.declare_dram_parameter("maskc", [128, 8], F32, isOutput=False)
    oldb_in = nc.declare_dram_parameter("olds_bin", [128, NT], F32, isOutput=False)
    oldu_in = nc.declare_dram_parameter("olds_un", [32, 4], F32, isOutput=False)
    oldn_in = nc.declare_dram_parameter("olds_null", [1, 4], F32, isOutput=False)

    out_binm = nc.declare_dram_parameter("out_binm", [128, NT], F32, isOutput=True)
    out_unm = nc.declare_dram_parameter("out_unm", [32, 4], F32, isOutput=True)
    out_nullm = nc.declare_dram_parameter("out_nullm", [1, 4], F32, isOutput=True)


    import contextlib

    with tile.TileContext(nc) as tc:
        with tc.tile_pool(name="cb", bufs=1) as cb, \
             tc.tile_pool(name="wk", bufs=1) as wk, \
             tc.tile_pool(name="ps", bufs=2, space="PSUM") as ps:
            _es = contextlib.ExitStack()
            ps2 = _es.enter_context(
                tc.tile_pool(name="ps2", bufs=1, space="PSUM"))
            _es2 = None

            # ---------- input DMAs first (small/latency-critical first) ----------
            akt = cb.tile([112, 72], F32)
            nc.sync.dma_start(akt[:], akt_in[:])
            okt = cb.tile([1, 24], F32)
            nc.sync.dma_start(okt[:], okt_in := ork_in[:])
            xu = cb.tile([128, 80], B16)
            nc.sync.dma_start(xu[:], xu_in[:])
            maskc = cb.tile([128, 8], F32)
            nc.sync.dma_start(maskc[:], mask_in[:])
            sel = cb.tile([32, 1152], B16)
            nc.sync.dma_start(sel[:], sel_in[:])
            oldb = cb.tile([128, NT], F32)
            nc.gpsimd.dma_start(oldb[:], oldb_in[:])
            oldu = cb.tile([32, 4], F32)
            nc.gpsimd.dma_start(oldu[:], oldu_in[:])
            oldn = cb.tile([1, 4], F32)
            nc.gpsimd.dma_start(oldn[:], oldn_in[:])
            x_all = cb.tile([128, NT * 32], B16)
            for h in range(4):
                nc.sync.dma_start(x_all[:, h * 256:(h + 1) * 256],
                                  x_all_in[:, h * 256:(h + 1) * 256])
            # channel-major replicas of batches 2,3 for the scalar-engine path
            xr = cb.tile([128, 2048], B16)
            for h in range(4):
                nc.gpsimd.dma_start(xr[:, h * 512:(h + 1) * 512],
                                    xr_in[:, h * 512:(h + 1) * 512])

            # ---------- phase A: softmax -> clamped gamma -> h = 1/gamma ----------
            e = wk.tile([112, 72], F32)
            nc.scalar.activation(e[:], akt[:], AF.Exp)
            eok = wk.tile([1, 24], F32)
            nc.scalar.activation(eok[:], okt[:], AF.Exp, scale=-1.0)
            e3 = e[:].rearrange("p (r m) -> p r m", m=3)
            bsum = wk.tile([112, 24], F32)
            nc.vector.tensor_tensor(bsum[:], e3[:, :, 1], e3[:, :, 2], op=ADD)
            stot = wk.tile([112, 24], F32)
            nc.vector.tensor_tensor(stot[:], e3[:, :, 0], bsum[:], op=ADD)
            gam = wk.tile([112, 24], F32)
            nc.vector.tensor_tensor(gam[:], e3[:, :, 0], e3[:, :, 1], op=SUB)
            rbs = wk.tile([112, 24], F32)
            nc.vector.reciprocal(rbs[:], bsum[:])
            nc.vector.tensor_tensor(gam[:], gam[:], rbs[:], op=MUL)
            # clamp |gamma| >= EPS (shift by EPS*sign so h = 1/gamma is bounded)
            gsh = wk.tile([112, 24], F32)
            nc.vector.tensor_scalar(gsh[:], gam[:], 0.0, None, op0=GE)
            nc.vector.tensor_scalar(gsh[:], gsh[:], 2.0 * EPS, -EPS,
                                    op0=MUL, op1=ADD)
            nc.vector.tensor_tensor(gam[:], gam[:], gsh[:], op=ADD)
            # per-partition gamma columns for the scalar-engine ln path:
            # gcol[c + 32*r4, r6] = gamma[rd = 4*r6 + r4, k = 80 + c]
            gambin = cb.tile([32, 24], F32)
            nc.scalar.dma_start(gambin[:], gam[80:112, :])
            gcol = cb.tile([128, 6], F32)
            for r4 in range(4):
                nc.vector.tensor_copy(
                    gcol[32 * r4:32 * (r4 + 1), :],
                    gambin[:].rearrange("p (a b) -> p a b", b=4)[:, :, r4])
            hrc = wk.tile([112, 24], F32)
            nc.vector.reciprocal(hrc[:], gam[:])

            # transpose h on-chip: hT[rd, k] (rows 24..31 / cols 112+ junk=1.0)
            hP = wk.tile([128, 32], F32)
            nc.vector.memset(hP[:], 1.0)
            nc.vector.tensor_copy(hP[0:112, 0:24], hrc[:])
            hT = cb.tile([32, 128], F32)
            for blk in range(4):
                nc.vector.transpose(hT[0:32, blk * 32:(blk + 1) * 32],
                                    hP[blk * 32:(blk + 1) * 32, 0:32])
            hTb = cb.tile([32, 128], B16)
            nc.vector.tensor_copy(hTb[:], hT[:])
            # gamma products over the pair-tree's stride groups: G8 = 1/prod h
            # cols: [0:2] nullary (j mod 2), [2:10] unary (half, j mod 4),
            #       [10:14] binary (j mod 4)
            g8p = wk.tile([24, 14], F32)
            nc.vector.tensor_reduce(
                g8p[:, 0:2].rearrange("p (j o) -> p j o", o=1),
                hT[0:24, 0:16].rearrange("p (e j) -> p j e", j=2),
                axis=AX, op=MUL)
            nc.vector.tensor_reduce(
                g8p[:, 2:10].rearrange("p (g o) -> p g o", o=1),
                hT[0:24, 16:80].rearrange("p (h e j) -> p h j e", h=2, j=4),
                axis=AX, op=MUL)
            nc.vector.tensor_reduce(
                g8p[:, 10:14].rearrange("p (j o) -> p j o", o=1),
                hT[0:24, 80:112].rearrange("p (e j) -> p j e", j=4),
                axis=AX, op=MUL)
            g8r = wk.tile([24, 14], F32)
            nc.vector.reciprocal(g8r[:], g8p[:])

            lnb = wk.tile([112, 24], F32)
            nc.scalar.activation(lnb[:], bsum[:], AF.Ln)
            lns = wk.tile([112, 24], F32)
            nc.scalar.activation(lns[:], stot[:], AF.Ln)
            nc.vector.tensor_tensor(lnb[:], lnb[:], lns[:], op=SUB)
            ones112 = cb.tile([112, 1], F32)
            nc.vector.memset(ones112[:], 1.0)
            psb = ps.tile([1, 24], F32, tag="pp")
            nc.tensor.matmul(psb[:], ones112[:], lnb[:], start=True, stop=True)
            psbS = cb.tile([1, 24], F32)
            nc.vector.tensor_copy(psbS[:], psb[:])
            # sigmoid(ork) = 1 / (1 + exp(-ork)); bA=exp(psb) deferred so the
            # ACT table order stays Exp -> Ln (conjunct lns) -> Exp
            sig0 = wk.tile([1, 24], F32)
            nc.vector.tensor_scalar(sig0[:], eok[:], 1.0, None, op0=ADD)
            nc.vector.reciprocal(sig0[:], sig0[:])

            # selector stack: sel24[:, r6*24 + rd] = 1 on rows (c, r4=rd%4)
            # iff rd//4 == r6 -> accumulate 6 chunks into one [24, .] psum
            F16 = mybir.dt.float16
            sel24 = cb.tile([128, 144], F16)
            nc.gpsimd.memset(sel24[:], 0.0)
            for r6 in range(6):
                for r4 in range(4):
                    nc.gpsimd.memset(
                        sel24[32 * r4:32 * (r4 + 1),
                              r6 * 24 + 4 * r6 + r4:r6 * 24 + 4 * r6 + r4 + 1],
                        1.0)
            ident24 = cb.tile([24, 24], B16)
            _make_identity(nc, ident24[:])

            # ---------- phase B: broadcast h / G8 consts via PE (bf16) ----------
            ones1 = cb.tile([1, 128], B16)
            nc.vector.memset(ones1[:], 1.0)

            # binary h: (rd, c32), k = 80+c
            g1 = cb.tile([1, 768], B16)
            nc.sync.dma_start(g1[:].rearrange("p (r c) -> p r c", r=24),
                              hTb[0:24, 80:112])
            gB = cb.tile([128, 768], B16)
            for h in range(2):
                pst = ps.tile([128, 384], F32, tag="pp")
                nc.tensor.matmul(pst[:], ones1[:], g1[:, h * 384:(h + 1) * 384],
                                 start=True, stop=True)
                nc.vector.tensor_copy(gB[:, h * 384:(h + 1) * 384], pst[:])

            # G8 products bcast: [bin 96 | un 192 | null 48] = 336
            g8f = cb.tile([1, 336], F32)
            nc.sync.dma_start(g8f[:, 0:96].rearrange("p (r c) -> p r c", r=24),
                              g8r[0:24, 10:14])
            nc.sync.dma_start(g8f[:, 96:288].rearrange("p (r c) -> p r c", r=24),
                              g8r[0:24, 2:10])
            nc.sync.dma_start(g8f[:, 288:336].rearrange("p (r c) -> p r c", r=24),
                              g8r[0:24, 0:2])
            ones1f = cb.tile([1, 128], F32)
            nc.vector.memset(ones1f[:], 1.0)
            G8b = cb.tile([128, 336], B16)
            pst = ps.tile([128, 336], F32, tag="pp")
            nc.tensor.matmul(pst[:], ones1f[:], g8f[:], start=True, stop=True)
            nc.vector.tensor_copy(G8b[:], pst[:])

            # unary h: (seg2, rd, c32), k = 16 + seg*32 + c
            u1 = cb.tile([1, 1536], B16)
            nc.sync.dma_start(u1[:].rearrange("p (r c) -> p r c", r=24),
                              hTb[0:24, 16:80])
            gUps = []
            for h in range(3):
                pst = ps2.tile([128, 512], F32, tag=f"gu{h}")
                nc.tensor.matmul(pst[:], ones1[:], u1[:, h * 512:(h + 1) * 512],
                                 start=True, stop=True)
                gUps.append(pst)

            # nullary h: (rd, c16), k = c
            n1 = cb.tile([1, 384], B16)
            nc.sync.dma_start(n1[:].rearrange("p (r c) -> p r c", r=24),
                              hTb[0:24, 0:16])
            gN = cb.tile([128, 384], B16)
            pst = ps.tile([128, 384], F32, tag="pp")
            nc.tensor.matmul(pst[:], ones1[:], n1[:], start=True, stop=True)
            nc.vector.tensor_copy(gN[:], pst[:])

            # ---------- phase C: unary/nullary factor pass (h-form) ----------
            emU = wk.tile([128, 1536], B16)
            # flat (s, rd, c): chunk boundaries vs the s-split of x
            def _emu(dst_lo, n_grp, x_lo, gsrc, src_lo):
                nc.vector.tensor_tensor(
                    emU[:, dst_lo:dst_lo + n_grp * 32]
                        .rearrange("p (g c) -> p g c", c=32),
                    xu[:, x_lo:x_lo + 32].unsqueeze(1)
                        .broadcast_to((128, n_grp, 32)),
                    gsrc[:][:, src_lo:src_lo + n_grp * 32]
                        .rearrange("p (g c) -> p g c", c=32), op=ADD)
            _emu(0, 16, 0, gUps[0], 0)
            _emu(512, 8, 0, gUps[1], 0)
            _emu(768, 8, 32, gUps[1], 256)
            _emu(1024, 16, 32, gUps[2], 0)
            _es.close()  # release gUps PSUM banks for the psln pool
            _es2 = contextlib.ExitStack()
            psl = _es2.enter_context(
                tc.tile_pool(name="psl", bufs=1, space="PSUM"))
            # pair-tree to prod-of-8, fold G8, tree to fu12 [128, 48]
            cur = emU[:].rearrange("p (g c) -> p g c", c=32)
            for w in (16, 8, 4):
                nxt = wk.tile([128, 48 * w], B16, tag=f"ut{w}")
                nc.vector.tensor_tensor(
                    nxt[:].rearrange("p (g c) -> p g c", c=w),
                    cur[:, :, 0:w], cur[:, :, w:2 * w], op=MUL)
                cur = nxt[:].rearrange("p (g c) -> p g c", c=w)
            p8u = wk.tile([128, 192], B16)
            nc.vector.tensor_tensor(
                p8u[:].rearrange("p (g c) -> p g c", c=4),
                cur, G8b[:, 96:288].rearrange("p (g c) -> p g c", c=4), op=MUL)
            cur = p8u[:].rearrange("p (g c) -> p g c", c=4)
            u2 = wk.tile([128, 96], B16)
            nc.vector.tensor_tensor(
                u2[:].rearrange("p (g c) -> p g c", c=2),
                cur[:, :, 0:2], cur[:, :, 2:4], op=MUL)
            fu12 = wk.tile([128, 48], B16)
            u2v = u2[:].rearrange("p (g c) -> p g c", c=2)
            nc.vector.tensor_tensor(fu12[:].unsqueeze(2),
                                    u2v[:, :, 0:1], u2v[:, :, 1:2], op=MUL)

            emN = wk.tile([128, 384], B16)
            nc.vector.tensor_tensor(
                emN[:].rearrange("p (r c) -> p r c", r=24),
                xu[:, 64:80].unsqueeze(1).broadcast_to((128, 24, 16)),
                gN[:].rearrange("p (r c) -> p r c", r=24), op=ADD)
            cur = emN[:].rearrange("p (g c) -> p g c", c=16)
            for w in (8, 4, 2):
                nxt = wk.tile([128, 24 * w], B16, tag=f"nt{w}")
                nc.vector.tensor_tensor(
                    nxt[:].rearrange("p (g c) -> p g c", c=w),
                    cur[:, :, 0:w], cur[:, :, w:2 * w], op=MUL)
                cur = nxt[:].rearrange("p (g c) -> p g c", c=w)
            p8n = wk.tile([128, 48], B16)
            nc.vector.tensor_tensor(
                p8n[:].rearrange("p (g c) -> p g c", c=2),
                cur, G8b[:, 288:336].rearrange("p (g c) -> p g c", c=2), op=MUL)
            f0g = wk.tile([128, 24], B16)
            nc.vector.tensor_tensor(
                f0g[:].unsqueeze(2),
                p8n[:].rearrange("p (g c) -> p g c", c=2)[:, :, 0:1],
                p8n[:].rearrange("p (g c) -> p g c", c=2)[:, :, 1:2], op=MUL)

            fu2f0 = wk.tile([128, 24], B16)
            nc.vector.tensor_tensor(fu2f0[:], fu12[:, 24:48], f0g[:], op=MUL)

            # ---------- phase D: per-b row broadcasts via PE ----------
            FU1B = cb.tile([128, 768], B16)
            FU2F0B = cb.tile([128, 96], B16)
            for b in range(BL):
                rhs1 = wk.tile([32, 24], B16, tag="rhs1")
                nc.vector.tensor_copy(rhs1[:], fu12[b * 32:(b + 1) * 32, 0:24])
                rhs2 = wk.tile([32, 24], B16, tag="rhs2")
                nc.vector.tensor_copy(rhs2[:], fu2f0[b * 32:(b + 1) * 32, :])
                psF = ps.tile([128, 192], F32, tag="pp")
                for t in range(8):
                    nc.tensor.matmul(psF[:, t * 24:(t + 1) * 24],
                                     sel[0:32, t * 128:(t + 1) * 128],
                                     rhs1[:], start=True, stop=True)
                nc.vector.tensor_copy(FU1B[:, b * 192:(b + 1) * 192], psF[:])
                psJ = ps.tile([128, 24], F32, tag="pp")
                nc.tensor.matmul(psJ[:], sel[0:32, 1024:1152],
                                 rhs2[:], start=True, stop=True)
                nc.vector.tensor_copy(FU2F0B[:, b * 24:(b + 1) * 24], psJ[:])

            # ---------- phase E-L2: batches 2,3 via scalar-engine log path ----------
            # ln(gamma*x + 1) on ACT (Ln table still loaded), segment-sum over
            # the 32 channels on PE into psln[rd, perm], exp later.
            psln = {}
            for bl in (2, 3):
                pl0 = psl.tile([24, 512], F32, tag=f"ln{bl}0")
                pl1 = psl.tile([24, 512], F32, tag=f"ln{bl}1")
                psln[bl] = [pl0, pl1]
            lnc_last = None
            for r6 in range(6):
                lnc = wk.tile([128, 2048], F16, tag=f"lnc{r6 % 4}")
                nc.scalar.activation(lnc[:], xr[:], AF.Ln, bias=1.0,
                                     scale=gcol[:, r6:r6 + 1])
                for bl in (2, 3):
                    for h in (0, 1):
                        nc.tensor.matmul(
                            psln[bl][h][:],
                            sel24[:, r6 * 24:(r6 + 1) * 24],
                            lnc[:, (bl - 2) * 1024 + h * 512:
                                (bl - 2) * 1024 + (h + 1) * 512],
                            start=(r6 == 0), stop=(r6 == 5))
                lnc_last = lnc

            # ---------- deferred: bA/sig (Exp table), OR-kernel bcast, PFOK ----------
            # dummy dep on the last ln chunk pins the Exp-table reload after
            # the Ln block (the scheduler may otherwise interleave and thrash
            # the ACT table)
            psbS2 = cb.tile([1, 24], F32)
            nc.vector.scalar_tensor_tensor(psbS2[:], lnc_last[0:1, 0:24], 0.0,
                                           psbS[:], op0=MUL, op1=ADD)
            bA = wk.tile([1, 24], F32)
            nc.scalar.activation(bA[:], psbS2[:], AF.Exp)
            sig = wk.tile([1, 24], F32)
            nc.vector.tensor_tensor(sig[:], sig0[:], bA[:], op=MUL)
            psO = ps.tile([128, 24], F32, tag="pp")
            nc.tensor.matmul(psO[:], ones1f[:], sig[:], start=True, stop=True)
            okmB = cb.tile([128, 192], B16)
            for t in range(8):
                nc.vector.tensor_scalar(okmB[:, t * 24:(t + 1) * 24], psO[:],
                                        maskc[:, t:t + 1], None, op0=MUL)

            # PFOK[p, (b,t,rd)] = FU1B * FU2F0B(bcast t) * okmB(bcast b)
            PFOK = cb.tile([128, 768], B16)
            nc.vector.tensor_tensor(
                PFOK[:].rearrange("p (b t r) -> p b t r", b=4, t=8),
                FU1B[:].rearrange("p (b t r) -> p b t r", b=4, t=8),
                FU2F0B[:].rearrange("p (b r) -> p b r", b=4)
                    .unsqueeze(2).broadcast_to((128, 4, 8, 24)), op=MUL)
            nc.vector.tensor_tensor(
                PFOK[:].rearrange("p (b t r) -> p b t r", b=4, t=8),
                PFOK[:].rearrange("p (b t r) -> p b t r", b=4, t=8),
                okmB[:].rearrange("p (t r) -> p t r", t=8)
                    .unsqueeze(1).broadcast_to((128, 4, 8, 24)), op=MUL)

            # exp(psln) -> conj, transpose back to perm-major via PE
            conj_sb = {}
            for bl in (2, 3):
                csb = wk.tile([24, 1024], B16, tag=f"cs{bl}")
                conj_sb[bl] = csb
                for h in (0, 1):
                    nc.scalar.activation(conj_sb[bl][:, h * 512:(h + 1) * 512],
                                         psln[bl][h][:], AF.Exp)

            # ---------- phase E-L1: batches 0,1 via h-form pair-trees ----------
            em = wk.tile([128, 2 * 6144], B16)
            t1 = wk.tile([128, 2 * 3072], B16)
            t2 = wk.tile([128, 2 * 1536], B16)
            t3 = wk.tile([128, 2 * 768], B16)
            t4 = wk.tile([128, 2 * 384], B16)
            cj = wk.tile([128, NT * 24], B16)
            for b in range(2):
                s = b % 2
                emb = em[:, s * 6144:(s + 1) * 6144]
                nc.vector.tensor_tensor(
                    emb.rearrange("p (k r c) -> p k r c", k=8, r=24),
                    x_all[:, b * 256:(b + 1) * 256]
                        .rearrange("p (k c) -> p k c", k=8)
                        .unsqueeze(2).broadcast_to((128, 8, 24, 32)),
                    gB[:].rearrange("p (r c) -> p r c", r=24)
                        .unsqueeze(1).broadcast_to((128, 8, 24, 32)), op=ADD)
                cur = emb.rearrange("p (g c) -> p g c", c=32)
                for w, tl in ((16, t1), (8, t2), (4, t3)):
                    dst = tl[:, s * 192 * w:(s + 1) * 192 * w].rearrange(
                        "p (g c) -> p g c", c=w)
                    nc.vector.tensor_tensor(dst, cur[:, :, 0:w],
                                            cur[:, :, w:2 * w], op=MUL)
                    cur = dst
                # fold G8 at the prod-of-8 stage to keep magnitudes bounded
                p8b = t3[:, s * 768:(s + 1) * 768]
                nc.vector.tensor_tensor(
                    p8b.rearrange("p (k n) -> p k n", k=8),
                    p8b.rearrange("p (k n) -> p k n", k=8),
                    G8b[:, 0:96].unsqueeze(1).broadcast_to((128, 8, 96)),
                    op=MUL)
                dst = t4[:, s * 384:(s + 1) * 384].rearrange(
                    "p (g c) -> p g c", c=2)
                nc.vector.tensor_tensor(dst, cur[:, :, 0:2], cur[:, :, 2:4],
                                        op=MUL)
                nc.vector.tensor_tensor(
                    cj[:, b * 192:(b + 1) * 192].unsqueeze(2),
                    dst[:, :, 0:1], dst[:, :, 1:2], op=MUL)

            # transpose conj[rd, perm] -> cj[perm, (k, rd)] for batches 2,3
            for bl in (2, 3):
                psT = ps.tile([128, 192], B16, tag="ppt")
                for k in range(8):
                    nc.tensor.transpose(psT[:, k * 24:(k + 1) * 24],
                                        conj_sb[bl][:, k * 128:(k + 1) * 128],
                                        ident24[:])
                nc.scalar.activation(cj[:, bl * 192:(bl + 1) * 192], psT[:],
                                     AF.Copy)

            # batched tail: conj * PFOK, probsum over d, pd assembly
            nc.vector.tensor_tensor(cj[:], cj[:], PFOK[:], op=MUL)
            gA = wk.tile([128, 768], B16)
            nc.vector.tensor_scalar(gA[:], cj[:], -1.0, 1.0, op0=MUL, op1=ADD)
            d1 = wk.tile([128, 384], B16)
            gv = gA[:].rearrange("p (g dd) -> p g dd", dd=8)
            nc.vector.tensor_tensor(d1[:].rearrange("p (g dd) -> p g dd", dd=4),
                                    gv[:, :, 0:4], gv[:, :, 4:8], op=MUL)
            d2 = wk.tile([128, 192], B16)
            d1v = d1[:].rearrange("p (g dd) -> p g dd", dd=4)
            nc.vector.tensor_tensor(d2[:].rearrange("p (g dd) -> p g dd", dd=2),
                                    d1v[:, :, 0:2], d1v[:, :, 2:4], op=MUL)
            pdA2 = wk.tile([128, 96], B16)
            d2v = d2[:].rearrange("p (g dd) -> p g dd", dd=2)
            nc.vector.tensor_tensor(pdA2[:].unsqueeze(2),
                                    d2v[:, :, 0:1], d2v[:, :, 1:2], op=MUL)
            # relayout (b, t, r) -> (r, b, t) while casting to fp32
            pdF = wk.tile([128, 96], F32)
            nc.vector.tensor_copy(
                pdF[:].rearrange("p (r b t) -> p r b t", r=3, b=4),
                pdA2[:].rearrange("p (b t r) -> p b t r", b=4, t=8)
                    .transpose([0, 3, 1, 2]))

            # ---------- phase F: merges (all on-chip) ----------
            # binary last channel (row layout: [128, NT]); pd r=2 block
            tb = wk.tile([128, NT], F32)
            nc.vector.tensor_scalar(tb[:], oldb[:], -1.0, 1.0, op0=MUL, op1=ADD)
            nc.vector.tensor_tensor(tb[:], tb[:], pdF[:, 64:96], op=MUL)
            nc.vector.tensor_scalar(tb[:], tb[:], -1.0, 1.0, op0=MUL, op1=ADD)
            nc.sync.dma_start(out_binm[:], tb[:])

            # transpose r=0 / r=1 pd blocks to [32 rows=(b,i8), 128=(i4,j)]
            r1T = wk.tile([32, 128], F32)
            r0T = wk.tile([32, 128], F32)
            for blk in range(4):
                nc.vector.transpose(r1T[0:32, blk * 32:(blk + 1) * 32],
                                    pdF[blk * 32:(blk + 1) * 32, 32:64])
                nc.vector.transpose(r0T[0:32, blk * 32:(blk + 1) * 32],
                                    pdF[blk * 32:(blk + 1) * 32, 0:32])

            # unary: product over j within each (b, i8, i4)
            pdu_t = wk.tile([32, 4], F32)
            nc.vector.tensor_reduce(
                pdu_t[:], r1T[:].rearrange("p (i4 j) -> p i4 j", i4=4),
                axis=AX, op=MUL)
            pdu = pdu_t[:]
            tu = wk.tile([32, 4], F32)
            nc.vector.tensor_scalar(tu[:], oldu[:], -1.0, 1.0, op0=MUL, op1=ADD)
            nc.vector.tensor_tensor(tu[:], tu[:], pdu, op=MUL)
            nc.vector.tensor_scalar(tu[:], tu[:], -1.0, 1.0, op0=MUL, op1=ADD)
            nc.sync.dma_start(out_unm[:], tu[:])

            # nullary: product over all (i, j) per b
            red0 = wk.tile([32, 1], F32)
            nc.vector.tensor_reduce(red0[:], r0T[:], axis=AX, op=MUL)
            # fold the remaining 32 partition values (b, i8) -> per-b products
            q = wk.tile([32, 32], F32)
            nc.vector.memset(q[:], 1.0)
            nc.vector.tensor_copy(q[:, 0:1], red0[:])
            qT = wk.tile([32, 32], F32)
            nc.vector.transpose(qT[:], q[:])
            pdn_t = wk.tile([1, 4], F32)
            nc.vector.tensor_reduce(
                pdn_t[:], qT[0:1, :].rearrange("p (b i8) -> p b i8", b=4),
                axis=AX, op=MUL)
            pdn = pdn_t[:]
            tn = wk.tile([1, 4], F32)
            nc.vector.tensor_scalar(tn[:], oldn[:], -1.0, 1.0, op0=MUL, op1=ADD)
            nc.vector.tensor_tensor(tn[:], tn[:], pdn, op=MUL)
            nc.vector.tensor_scalar(tn[:], tn[:], -1.0, 1.0, op0=MUL, op1=ADD)
            nc.sync.dma_start(out_nullm[:], tn[:])
            _es2.close()

    nc.compile()
    return nc


def _host_prep(nullary_preds, unary_preds, binary_preds, and_kernel, or_kernel):
    """Build per-core input maps (sharding + layout prep only)."""
    null_ = np.asarray(nullary_preds, np.float32)
    un = np.asarray(unary_preds, np.float32)
    bi = np.asarray(binary_preds, np.float32)
    ak = np.asarray(and_kernel, np.float32)
    ok = np.asarray(or_kernel, np.float32)

    I, J = np.meshgrid(np.arange(N), np.arange(N), indexing="ij")
    off = I != J
    Jm = J - (J > I)
    Im = I - (I > J)

    binP = np.zeros((B, N, N, P2), np.float32)
    binP[:, off] = bi[:, I[off], Jm[off]]
    binT = np.zeros((B, N, N, P2), np.float32)
    binT[:, off] = bi[:, J[off], Im[off]]
    binPT = np.concatenate([binP, binT], axis=-1)          # [B,32,32,32]

    # row-tile layout: x_all[core][p, k=(b,t), c] = binPT[4c+b, t*128+p, c]
    xg = binPT.reshape(NCORE, BL, 8, 128, 32)
    x_all = np.ascontiguousarray(xg.transpose(0, 3, 1, 2, 4)
                                 ).reshape(NCORE, 128, NT * 32).astype(BF)
    # channel-major replicas of local batches 2,3 for the ACT-ln path:
    # xr[core][c + 32*r4, (bl-2)*1024 + i*32 + j] = binPT[core*4+bl, i, j, c]
    xrg = binPT.reshape(NCORE, BL, 1024, 32)[:, 2:4]        # [8, 2, 1024, 32]
    xrg = xrg.transpose(0, 3, 1, 2).reshape(NCORE, 1, 32, 2048)
    xr = np.broadcast_to(xrg, (NCORE, 4, 32, 2048)).reshape(
        NCORE, 128, 2048).astype(BF)
    olds_bin = np.ascontiguousarray(
        binP[..., 15].reshape(NCORE, BL, 8, 128).transpose(0, 3, 1, 2)
    ).reshape(NCORE, 128, NT).astype(np.float32)

    # unary pass rows (b, i): [u | u | n]
    xun = np.concatenate(
        [un, un, np.broadcast_to(null_[:, None, :], (B, N, P0))], axis=-1)
    xu = xun.reshape(NCORE, 128, 80).astype(BF)
    # rows (b, i8), cols i4 : out_unm[q=(b*8+i//4), i%4]
    olds_un = un[..., 31].reshape(NCORE, 4, 8, 4).reshape(NCORE, 32, 4).astype(np.float32)
    olds_null = null_[:, 15].reshape(NCORE, 1, 4).astype(np.float32)

    akT = np.ascontiguousarray(ak.transpose(2, 0, 1, 3)).reshape(112, 72)
    ork = ok.reshape(1, 24).astype(np.float32)

    p = np.arange(128)
    t = np.arange(8)
    selT = (np.arange(32)[:, None, None] == (t[None, :, None] * 4 + p[None, None, :] // 32))
    selJ = (np.arange(32)[:, None] == (p[None, :] % 32))
    selcat = np.concatenate([selT.reshape(32, 1024), selJ], axis=1).astype(BF)
    maskc = ((p[:, None] % 32) != (t[None, :] * 4 + p[:, None] // 32)
             ).astype(np.float32)

    in_maps = []
    for c in range(NCORE):
        in_maps.append({
            "x_all": x_all[c],
            "xr": np.ascontiguousarray(xr[c]),
            "xu": xu[c],
            "akt": akT,
            "ork": ork,
            "selcat": selcat,
            "maskc": maskc,
            "olds_bin": olds_bin[c],
            "olds_un": olds_un[c],
            "olds_null": olds_null[c],
        })
    return in_maps


def _assemble(results, nullary_preds, unary_preds, binary_preds):
    null_ = np.asarray(nullary_preds, np.float32).copy()
    un = np.asarray(unary_preds, np.float32).copy()
    bi = np.asarray(binary_preds, np.float32).copy()

    I, J = np.meshgrid(np.arange(N), np.arange(N), indexing="ij")
    off = I != J
    Jm = J - (J > I)

    for c in range(NCORE):
        r = results[c]
        # out_binm [128, NT=(b,t)] -> rows[b, t*128+p]
        ob = r["out_binm"].reshape(128, BL, 8).transpose(1, 2, 0).reshape(BL, N, N)
        for bl in range(BL):
            b = c * BL + bl
            bi[b, I[off], Jm[off], 15] = ob[bl][off]
        un[c * BL:(c + 1) * BL, :, 31] = r["out_unm"].reshape(BL, 8, 4).reshape(BL, N)
        null_[c * BL:(c + 1) * BL, 15] = r["out_nullm"].reshape(BL)

    return np.concatenate(
        [null_, un.reshape(B, -1), bi.reshape(B, -1)], axis=-1)


def kernel(nullary_preds, unary_preds, binary_preds, and_kernel, or_kernel):
    from concourse.bass_utils import run_bass_kernel_spmd

    if "nc" not in _CACHE:
        _CACHE["nc"] = _build()
    nc = _CACHE["nc"]

    in_maps = _host_prep(nullary_preds, unary_preds, binary_preds,
                         and_kernel, or_kernel)
    res = run_bass_kernel_spmd(nc, in_maps, list(range(NCORE)))
    return _assemble(res.results, nullary_preds, unary_preds, binary_preds)


if __name__ == "__main__":
    import reference as ref
    ins = {k: np.asarray(v) for k, v in ref.setup_inputs().items()}
    out = kernel(**ins)
    print("kernel out:", out.shape, out.dtype)



# revision 45
# speedup vs baseline: 1.0276x; 1.0276x over previous
"""Trainium2 Bass kernel for nn_DNFLayer (fuzzy DNF layer).

Strategy
--------
Data-parallel over batch B=32 across 8 cores (4 batches/core). Per core the
(i, j) permutation grid is padded to the full 32x32 grid (diagonal masked via
the OR-kernel broadcast), giving 4096 rows = 32 row-tiles of 128 partitions.

The conjunct product over the 112 inputs is factorized per permutation
(i, j):  conj = F0(b) * FU1(b,i) * FU2(b,j) * FB1(b,i,j) * FB2(b,j,i),
each factor a product of per-channel affine terms (alpha*x + beta), with
(alpha, beta) derived on-device from softmax(and_kernel); the per-(r,d) beta
products fold into the OR-kernel broadcast via Ln / matmul column-sum / Exp.

The per-permutation binary work (32 channels x 24 (r,d) x 1024 perms x 4
batches) is split across engines to balance the NeuronCore:

* Batches 0,1 (DVE path): "h-form" evaluation
  prod(gamma x + 1) = prod(clamped gamma) * prod(x + 1/gamma), i.e. one
  bf16 tensor_tensor ADD (2x DVE mode) + a pairwise product tree, with the
  gamma-product G8 folded in at the product-of-8 stage to keep bf16
  magnitudes bounded (gamma is clamped to |gamma| >= 2e-4 first; the shift
  changes each factor by <= 2e-4 which is far inside the accuracy budget).

* Batches 2,3 (ACT/PE log path): the Scalar engine computes
  ln(gamma*x + 1) directly from a channel-major replica of x (gamma as a
  per-partition scale vector, bias=1), the PE segment-sums the 32 channels
  per (r,d) with 0/1 selector matmuls accumulated in PSUM, ACT exponentiates
  the PSUM, and the PE transposes conj[rd, perm] back to perm-major via
  identity matmuls. ln values are fp16 (exact enough: abs err ~2^-11 per
  term) so all matmuls are single-pass.

Both paths produce the same cj[perm, (k, rd)] tile; a batched tail applies
PFOK (unary/nullary factors x sigmoid(or_kernel) x diagonal mask), the
probabilistic-sum trees over disjuncts, and fp32 merges with the residual
inputs. The ACT table sequence is kept to Exp -> Ln -> Exp (3 loads) by
deferring bA = exp(sum ln beta) until after all Ln work.
"""

import numpy as np
import ml_dtypes

BF = ml_dtypes.bfloat16
B, N, P0, P1, P2, R, D = 32, 32, 16, 32, 16, 3, 8
RD = R * D              # 24
NCORE = 8
BL = B // NCORE         # 4 batches per core
NT = BL * 8             # 32 row-tiles of 128 per core

_CACHE = {}


def _build():
    import concourse.tile as tile
    from concourse import mybir, bacc

    from concourse.masks import make_identity as _make_identity

    F32 = mybir.dt.float32
    B16 = mybir.dt.bfloat16
    MUL = mybir.AluOpType.mult
    ADD = mybir.AluOpType.add
    SUB = mybir.AluOpType.subtract
    GE = mybir.AluOpType.is_ge
    AX = mybir.AxisListType.X
    AF = mybir.ActivationFunctionType
    EPS = 2e-4

    nc = bacc.Bacc("TRN2", target_bir_lowering=False, debug=False,
                   num_devices=NCORE)

    # ---- parameters (per-core shards / replicated constants) ----
    x_all_in = nc.declare_dram_parameter("x_all", [128, NT * 32], B16, isOutput=False)
    xr_in = nc.declare_dram_parameter("xr", [128, 2048], B16, isOutput=False)
    xu_in = nc.declare_dram_parameter("xu", [128, 80], B16, isOutput=False)
    akt_in = nc.declare_dram_parameter("akt", [112, 72], F32, isOutput=False)
    ork_in = nc.declare_dram_parameter("ork", [1, 24], F32, isOutput=False)
    sel_in = nc.declare_dram_parameter("selcat", [32, 1152], B16, isOutput=False)
    mask_in = nc.declare_dram_parameter("maskc", [128, 8], F32, isOutput=False)
    oldb_in = nc.declare_dram_parameter("olds_bin", [128, NT], F32, isOutput=False)
    oldu_in = nc.declare_dram_parameter("olds_un", [32, 4], F32, isOutput=False)
    oldn_in = nc.declare_dram_parameter("olds_null", [1, 4], F32, isOutput=False)

    out_binm = nc.declare_dram_parameter("out_binm", [128, NT], F32, isOutput=True)
    out_unm = nc.declare_dram_parameter("out_unm", [32, 4], F32, isOutput=True)
    out_nullm = nc.declare_dram_parameter("out_nullm", [1, 4], F32, isOutput=True)


    import contextlib

    with tile.TileContext(nc) as tc:
        with tc.tile_pool(name="cb", bufs=1) as cb, \
             tc.tile_pool(name="wk", bufs=1) as wk, \
             tc.tile_pool(name="ps", bufs=2, space="PSUM") as ps:
            _es = contextlib.ExitStack()
            ps2 = _es.enter_context(
                tc.tile_pool(name="ps2", bufs=1, space="PSUM"))
            _es2 = None

            # ---------- input DMAs first (small/latency-critical first) ----------
            akt = cb.tile([112, 72], F32)
            nc.sync.dma_start(akt[:], akt_in[:])
            okt = cb.tile([1, 24], F32)
            nc.sync.dma_start(okt[:], okt_in := ork_in[:])
            xu = cb.tile([128, 80], B16)
            nc.sync.dma_start(xu[:], xu_in[:])
            maskc = cb.tile([128, 8], F32)
            nc.sync.dma_start(maskc[:], mask_in[:])
            sel = cb.tile([32, 1152], B16)
            nc.sync.dma_start(sel[:], sel_in[:])
            oldb = cb.tile([128, NT], F32)
            nc.gpsimd.dma_start(oldb[:], oldb_in[:])
            oldu = cb.tile([32, 4], F32)
            nc.gpsimd.dma_start(oldu[:], oldu_in[:])
            oldn = cb.tile([1, 4], F32)
            nc.gpsimd.dma_start(oldn[:], oldn_in[:])
            x_all = cb.tile([128, NT * 32], B16)
            for h in range(4):
                nc.sync.dma_start(x_all[:, h * 256:(h + 1) * 256],
                                  x_all_in[:, h * 256:(h + 1) * 256])
            # channel-major replicas of batches 2,3 for the scalar-engine path
            xr = cb.tile([128, 2048], B16)
            for h in range(4):
                nc.gpsimd.dma_start(xr[:, h * 512:(h + 1) * 512],
                                    xr_in[:, h * 512:(h + 1) * 512])

            # ---------- phase A: softmax -> clamped gamma -> h = 1/gamma ----------
            e = wk.tile([112, 72], F32)
            nc.scalar.activation(e[:], akt[:], AF.Exp)
            eok = wk.tile([1, 24], F32)
            nc.scalar.activation(eok[:], okt[:], AF.Exp, scale=-1.0)
            e3 = e[:].rearrange("p (r m) -> p r m", m=3)
            bsum = wk.tile([112, 24], F32)
            nc.vector.tensor_tensor(bsum[:], e3[:, :, 1], e3[:, :, 2], op=ADD)
            stot = wk.tile([112, 24], F32)
            nc.vector.tensor_tensor(stot[:], e3[:, :, 0], bsum[:], op=ADD)
            gam = wk.tile([112, 24], F32)
            nc.vector.tensor_tensor(gam[:], e3[:, :, 0], e3[:, :, 1], op=SUB)
            rbs = wk.tile([112, 24], F32)
            nc.vector.reciprocal(rbs[:], bsum[:])
            nc.vector.tensor_tensor(gam[:], gam[:], rbs[:], op=MUL)
            # clamp |gamma| >= EPS (shift by EPS*sign so h = 1/gamma is bounded)
            gsh = wk.tile([112, 24], F32)
            nc.vector.tensor_scalar(gsh[:], gam[:], 0.0, None, op0=GE)
            nc.vector.tensor_scalar(gsh[:], gsh[:], 2.0 * EPS, -EPS,
                                    op0=MUL, op1=ADD)
            nc.vector.tensor_tensor(gam[:], gam[:], gsh[:], op=ADD)
            # per-partition gamma columns for the scalar-engine ln path:
            # gcol[c + 32*r4, r6] = gamma[rd = 4*r6 + r4, k = 80 + c]
            gambin = cb.tile([32, 24], F32)
            nc.scalar.dma_start(gambin[:], gam[80:112, :])
            gcol = cb.tile([128, 6], F32)
            for r4 in range(4):
                nc.vector.tensor_copy(
                    gcol[32 * r4:32 * (r4 + 1), :],
                    gambin[:].rearrange("p (a b) -> p a b", b=4)[:, :, r4])
            hrc = wk.tile([112, 24], F32)
            nc.vector.reciprocal(hrc[:], gam[:])
            hrcb = wk.tile([112, 24], B16)
            nc.vector.tensor_copy(hrcb[:], hrc[:])

            # transpose h on-chip: hT[rd, k] (rows 24..31 / cols 112+ junk=1.0)
            hP = wk.tile([128, 32], F32)
            nc.vector.memset(hP[:], 1.0)
            nc.vector.tensor_copy(hP[0:112, 0:24], hrc[:])
            hT = cb.tile([32, 128], F32)
            for blk in range(4):
                nc.vector.transpose(hT[0:32, blk * 32:(blk + 1) * 32],
                                    hP[blk * 32:(blk + 1) * 32, 0:32])
            # gamma products over the pair-tree's stride groups: G8 = 1/prod h
            # cols: [0:2] nullary (j mod 2), [2:10] unary (half, j mod 4),
            #       [10:14] binary (j mod 4)
            g8p = wk.tile([24, 14], F32)
            nc.vector.tensor_reduce(
                g8p[:, 0:2].rearrange("p (j o) -> p j o", o=1),
                hT[0:24, 0:16].rearrange("p (e j) -> p j e", j=2),
                axis=AX, op=MUL)
            nc.vector.tensor_reduce(
                g8p[:, 2:10].rearrange("p (g o) -> p g o", o=1),
                hT[0:24, 16:80].rearrange("p (h e j) -> p h j e", h=2, j=4),
                axis=AX, op=MUL)
            nc.vector.tensor_reduce(
                g8p[:, 10:14].rearrange("p (j o) -> p j o", o=1),
                hT[0:24, 80:112].rearrange("p (e j) -> p j e", j=4),
                axis=AX, op=MUL)
            g8r = wk.tile([24, 14], F32)
            nc.vector.reciprocal(g8r[:], g8p[:])

            lnb = wk.tile([112, 24], F32)
            nc.scalar.activation(lnb[:], bsum[:], AF.Ln)
            lns = wk.tile([112, 24], F32)
            nc.scalar.activation(lns[:], stot[:], AF.Ln)
            nc.vector.tensor_tensor(lnb[:], lnb[:], lns[:], op=SUB)
            ones112 = cb.tile([112, 1], F32)
            nc.vector.memset(ones112[:], 1.0)
            psb = ps.tile([1, 24], F32, tag="pp")
            nc.tensor.matmul(psb[:], ones112[:], lnb[:], start=True, stop=True)
            psbS = cb.tile([1, 24], F32)
            nc.vector.tensor_copy(psbS[:], psb[:])
            # sigmoid(ork) = 1 / (1 + exp(-ork)); bA=exp(psb) deferred so the
            # ACT table order stays Exp -> Ln (conjunct lns) -> Exp
            sig0 = wk.tile([1, 24], F32)
            nc.vector.tensor_scalar(sig0[:], eok[:], 1.0, None, op0=ADD)
            nc.vector.reciprocal(sig0[:], sig0[:])

            # selector stack: sel24[:, r6*24 + rd] = 1 on rows (c, r4=rd%4)
            # iff rd//4 == r6 -> accumulate 6 chunks into one [24, .] psum
            F16 = mybir.dt.float16
            sel24 = cb.tile([128, 144], F16)
            nc.gpsimd.memset(sel24[:], 0.0)
            for r6 in range(6):
                for r4 in range(4):
                    nc.gpsimd.memset(
                        sel24[32 * r4:32 * (r4 + 1),
                              r6 * 24 + 4 * r6 + r4:r6 * 24 + 4 * r6 + r4 + 1],
                        1.0)
            ident24 = cb.tile([24, 24], B16)
            _make_identity(nc, ident24[:])
            ident128 = cb.tile([128, 128], B16)
            _make_identity(nc, ident128[:])
            # merge complements (1 - old), hoisted out of the serial tail
            obC = cb.tile([128, NT], F32)
            nc.vector.tensor_scalar(obC[:], oldb[:], -1.0, 1.0, op0=MUL, op1=ADD)
            ouC = cb.tile([32, 4], F32)
            nc.vector.tensor_scalar(ouC[:], oldu[:], -1.0, 1.0, op0=MUL, op1=ADD)
            onC = cb.tile([1, 4], F32)
            nc.vector.tensor_scalar(onC[:], oldn[:], -1.0, 1.0, op0=MUL, op1=ADD)

            # ---------- phase B: broadcast h / G8 consts via PE (bf16) ----------
            ones1 = cb.tile([1, 128], B16)
            nc.vector.memset(ones1[:], 1.0)

            # binary h: (rd, c32), k = 80+c
            g1 = cb.tile([1, 768], B16)
            nc.sync.dma_start(g1[:].rearrange("p (c r) -> p c r", c=32),
                              hrcb[80:112, :])
            g1v = g1[:].rearrange("p (c r) -> p r c", c=32)
            gB = cb.tile([128, 768], B16)
            for h in range(2):
                pst = ps.tile([128, 384], F32, tag="pp")
                nc.tensor.matmul(pst[:], ones1[:], g1v[:, h * 12:(h + 1) * 12, :],
                                 start=True, stop=True)
                nc.vector.tensor_copy(gB[:, h * 384:(h + 1) * 384], pst[:])

            # G8 products bcast: [bin 96 | un 192 | null 48] = 336
            g8f = cb.tile([1, 336], F32)
            nc.sync.dma_start(g8f[:, 0:96].rearrange("p (r c) -> p r c", r=24),
                              g8r[0:24, 10:14])
            nc.sync.dma_start(g8f[:, 96:288].rearrange("p (r c) -> p r c", r=24),
                              g8r[0:24, 2:10])
            nc.sync.dma_start(g8f[:, 288:336].rearrange("p (r c) -> p r c", r=24),
                              g8r[0:24, 0:2])
            ones1f = cb.tile([1, 128], F32)
            nc.vector.memset(ones1f[:], 1.0)
            G8b = cb.tile([128, 336], B16)
            pst = ps.tile([128, 336], F32, tag="pp")
            nc.tensor.matmul(pst[:], ones1f[:], g8f[:], start=True, stop=True)
            nc.vector.tensor_copy(G8b[:], pst[:])

            # unary h: (seg2, rd, c32), k = 16 + seg*32 + c
            u1 = cb.tile([1, 1536], B16)
            nc.sync.dma_start(u1[:].rearrange("p (c r) -> p c r", c=64),
                              hrcb[16:80, :])
            u1v = u1[:].rearrange("p (c r) -> p r c", c=64)
            gUps = []
            for h in range(3):
                pst = ps2.tile([128, 512], F32, tag=f"gu{h}")
                nc.tensor.matmul(pst[:], ones1[:], u1v[:, h * 8:(h + 1) * 8, :],
                                 start=True, stop=True)
                gUps.append(pst)

            # nullary h: (rd, c16), k = c
            n1 = cb.tile([1, 384], B16)
            nc.sync.dma_start(n1[:].rearrange("p (c r) -> p c r", c=16),
                              hrcb[0:16, :])
            n1v = n1[:].rearrange("p (c r) -> p r c", c=16)
            gN = cb.tile([128, 384], B16)
            pst = ps.tile([128, 384], F32, tag="pp")
            nc.tensor.matmul(pst[:], ones1[:], n1v[:], start=True, stop=True)
            nc.vector.tensor_copy(gN[:], pst[:])

            # ---------- phase C: unary/nullary factor pass (h-form) ----------
            emU = wk.tile([128, 1536], B16)
            # flat (s, rd, c): chunk boundaries vs the s-split of x
            def _emu(dst_lo, n_grp, x_lo, gsrc, src_lo):
                nc.vector.tensor_tensor(
                    emU[:, dst_lo:dst_lo + n_grp * 32]
                        .rearrange("p (g c) -> p g c", c=32),
                    xu[:, x_lo:x_lo + 32].unsqueeze(1)
                        .broadcast_to((128, n_grp, 32)),
                    gsrc[:][:, src_lo:src_lo + n_grp * 32]
                        .rearrange("p (g c) -> p g c", c=32), op=ADD)
            _emu(0, 16, 0, gUps[0], 0)
            _emu(512, 8, 0, gUps[1], 0)
            _emu(768, 8, 32, gUps[1], 256)
            _emu(1024, 16, 32, gUps[2], 0)
            _es.close()  # release gUps PSUM banks for the psln pool
            _es2 = contextlib.ExitStack()
            psl = _es2.enter_context(
                tc.tile_pool(name="psl", bufs=1, space="PSUM"))
            # pair-tree to prod-of-8, fold G8, tree to fu12 [128, 48]
            cur = emU[:].rearrange("p (g c) -> p g c", c=32)
            for w in (16, 8, 4):
                nxt = wk.tile([128, 48 * w], B16, tag=f"ut{w}")
                nc.vector.tensor_tensor(
                    nxt[:].rearrange("p (g c) -> p g c", c=w),
                    cur[:, :, 0:w], cur[:, :, w:2 * w], op=MUL)
                cur = nxt[:].rearrange("p (g c) -> p g c", c=w)
            p8u = wk.tile([128, 192], B16)
            nc.vector.tensor_tensor(
                p8u[:].rearrange("p (g c) -> p g c", c=4),
                cur, G8b[:, 96:288].rearrange("p (g c) -> p g c", c=4), op=MUL)
            cur = p8u[:].rearrange("p (g c) -> p g c", c=4)
            u2 = wk.tile([128, 96], B16)
            nc.vector.tensor_tensor(
                u2[:].rearrange("p (g c) -> p g c", c=2),
                cur[:, :, 0:2], cur[:, :, 2:4], op=MUL)
            fu12 = wk.tile([128, 48], B16)
            u2v = u2[:].rearrange("p (g c) -> p g c", c=2)
            nc.vector.tensor_tensor(fu12[:].unsqueeze(2),
                                    u2v[:, :, 0:1], u2v[:, :, 1:2], op=MUL)

            emN = wk.tile([128, 384], B16)
            nc.vector.tensor_tensor(
                emN[:].rearrange("p (r c) -> p r c", r=24),
                xu[:, 64:80].unsqueeze(1).broadcast_to((128, 24, 16)),
                gN[:].rearrange("p (r c) -> p r c", r=24), op=ADD)
            cur = emN[:].rearrange("p (g c) -> p g c", c=16)
            for w in (8, 4, 2):
                nxt = wk.tile([128, 24 * w], B16, tag=f"nt{w}")
                nc.vector.tensor_tensor(
                    nxt[:].rearrange("p (g c) -> p g c", c=w),
                    cur[:, :, 0:w], cur[:, :, w:2 * w], op=MUL)
                cur = nxt[:].rearrange("p (g c) -> p g c", c=w)
            p8n = wk.tile([128, 48], B16)
            nc.vector.tensor_tensor(
                p8n[:].rearrange("p (g c) -> p g c", c=2),
                cur, G8b[:, 288:336].rearrange("p (g c) -> p g c", c=2), op=MUL)
            f0g = wk.tile([128, 24], B16)
            nc.vector.tensor_tensor(
                f0g[:].unsqueeze(2),
                p8n[:].rearrange("p (g c) -> p g c", c=2)[:, :, 0:1],
                p8n[:].rearrange("p (g c) -> p g c", c=2)[:, :, 1:2], op=MUL)

            fu2f0 = wk.tile([128, 24], B16)
            nc.vector.tensor_tensor(fu2f0[:], fu12[:, 24:48], f0g[:], op=MUL)

            # ---------- phase D: per-b row broadcasts via PE ----------
            FU1B = cb.tile([128, 768], B16)
            FU2F0B = cb.tile([128, 96], B16)
            for b in range(BL):
                rhs1 = wk.tile([32, 24], B16, tag="rhs1")
                nc.vector.tensor_copy(rhs1[:], fu12[b * 32:(b + 1) * 32, 0:24])
                rhs2 = wk.tile([32, 24], B16, tag="rhs2")
                nc.vector.tensor_copy(rhs2[:], fu2f0[b * 32:(b + 1) * 32, :])
                psF = ps.tile([128, 192], F32, tag="pp")
                for t in range(8):
                    nc.tensor.matmul(psF[:, t * 24:(t + 1) * 24],
                                     sel[0:32, t * 128:(t + 1) * 128],
                                     rhs1[:], start=True, stop=True)
                nc.vector.tensor_copy(FU1B[:, b * 192:(b + 1) * 192], psF[:])
                psJ = ps.tile([128, 24], F32, tag="pp")
                nc.tensor.matmul(psJ[:], sel[0:32, 1024:1152],
                                 rhs2[:], start=True, stop=True)
                nc.vector.tensor_copy(FU2F0B[:, b * 24:(b + 1) * 24], psJ[:])

            # ---------- phase E-L2: batches 2,3 via scalar-engine log path ----------
            # ln(gamma*x + 1) on ACT (Ln table still loaded), segment-sum over
            # the 32 channels on PE into psln[rd, perm], exp later.
            psln = {}
            for bl in (2, 3):
                pl0 = psl.tile([24, 512], F32, tag=f"ln{bl}0")
                pl1 = psl.tile([24, 512], F32, tag=f"ln{bl}1")
                psln[bl] = [pl0, pl1]
            lnc_last = None
            for r6 in range(6):
                lnc = wk.tile([128, 2048], F16, tag=f"lnc{r6 % 4}")
                nc.scalar.activation(lnc[:], xr[:], AF.Ln, bias=1.0,
                                     scale=gcol[:, r6:r6 + 1])
                for bl in (2, 3):
                    for h in (0, 1):
                        nc.tensor.matmul(
                            psln[bl][h][:],
                            sel24[:, r6 * 24:(r6 + 1) * 24],
                            lnc[:, (bl - 2) * 1024 + h * 512:
                                (bl - 2) * 1024 + (h + 1) * 512],
                            start=(r6 == 0), stop=(r6 == 5))
                lnc_last = lnc

            # ---------- deferred: bA/sig (Exp table), OR-kernel bcast, PFOK ----------
            # dummy dep on the last ln chunk pins the Exp-table reload after
            # the Ln block (the scheduler may otherwise interleave and thrash
            # the ACT table)
            psbS2 = cb.tile([1, 24], F32)
            nc.vector.scalar_tensor_tensor(psbS2[:], lnc_last[0:1, 0:24], 0.0,
                                           psbS[:], op0=MUL, op1=ADD)
            bA = wk.tile([1, 24], F32)
            nc.scalar.activation(bA[:], psbS2[:], AF.Exp)
            sig = wk.tile([1, 24], F32)
            nc.vector.tensor_tensor(sig[:], sig0[:], bA[:], op=MUL)
            psO = ps.tile([128, 24], F32, tag="pp")
            nc.tensor.matmul(psO[:], ones1f[:], sig[:], start=True, stop=True)
            okmB = cb.tile([128, 192], B16)
            for t in range(8):
                nc.vector.tensor_scalar(okmB[:, t * 24:(t + 1) * 24], psO[:],
                                        maskc[:, t:t + 1], None, op0=MUL)

            # PFOK[p, (b,t,rd)] = FU1B * FU2F0B(bcast t) * okmB(bcast b)
            PFOK = cb.tile([128, 768], B16)
            nc.vector.tensor_tensor(
                PFOK[:].rearrange("p (b t r) -> p b t r", b=4, t=8),
                FU1B[:].rearrange("p (b t r) -> p b t r", b=4, t=8),
                FU2F0B[:].rearrange("p (b r) -> p b r", b=4)
                    .unsqueeze(2).broadcast_to((128, 4, 8, 24)), op=MUL)
            nc.vector.tensor_tensor(
                PFOK[:].rearrange("p (b t r) -> p b t r", b=4, t=8),
                PFOK[:].rearrange("p (b t r) -> p b t r", b=4, t=8),
                okmB[:].rearrange("p (t r) -> p t r", t=8)
                    .unsqueeze(1).broadcast_to((128, 4, 8, 24)), op=MUL)

            # exp(psln) -> conj, transpose back to perm-major via PE
            conj_sb = {}
            for bl in (2, 3):
                csb = wk.tile([24, 1024], B16, tag=f"cs{bl}")
                conj_sb[bl] = csb
                for h in (0, 1):
                    nc.scalar.activation(conj_sb[bl][:, h * 512:(h + 1) * 512],
                                         psln[bl][h][:], AF.Exp)

            # ---------- phase E-L1: batches 0,1 via h-form pair-trees ----------
            em = wk.tile([128, 2 * 6144], B16)
            t1 = wk.tile([128, 2 * 3072], B16)
            t2 = wk.tile([128, 2 * 1536], B16)
            t3 = wk.tile([128, 2 * 768], B16)
            t4 = wk.tile([128, 2 * 384], B16)
            cj = wk.tile([128, NT * 24], B16)
            for b in range(2):
                s = b % 2
                emb = em[:, s * 6144:(s + 1) * 6144]
                nc.vector.tensor_tensor(
                    emb.rearrange("p (k r c) -> p k r c", k=8, r=24),
                    x_all[:, b * 256:(b + 1) * 256]
                        .rearrange("p (k c) -> p k c", k=8)
                        .unsqueeze(2).broadcast_to((128, 8, 24, 32)),
                    gB[:].rearrange("p (r c) -> p r c", r=24)
                        .unsqueeze(1).broadcast_to((128, 8, 24, 32)), op=ADD)
                cur = emb.rearrange("p (g c) -> p g c", c=32)
                for w, tl in ((16, t1), (8, t2), (4, t3)):
                    dst = tl[:, s * 192 * w:(s + 1) * 192 * w].rearrange(
                        "p (g c) -> p g c", c=w)
                    nc.vector.tensor_tensor(dst, cur[:, :, 0:w],
                                            cur[:, :, w:2 * w], op=MUL)
                    cur = dst
                # fold G8 at the prod-of-8 stage to keep magnitudes bounded
                p8b = t3[:, s * 768:(s + 1) * 768]
                nc.vector.tensor_tensor(
                    p8b.rearrange("p (k n) -> p k n", k=8),
                    p8b.rearrange("p (k n) -> p k n", k=8),
                    G8b[:, 0:96].unsqueeze(1).broadcast_to((128, 8, 96)),
                    op=MUL)
                dst = t4[:, s * 384:(s + 1) * 384].rearrange(
                    "p (g c) -> p g c", c=2)
                nc.vector.tensor_tensor(dst, cur[:, :, 0:2], cur[:, :, 2:4],
                                        op=MUL)
                nc.vector.tensor_tensor(
                    cj[:, b * 192:(b + 1) * 192].unsqueeze(2),
                    dst[:, :, 0:1], dst[:, :, 1:2], op=MUL)

            # transpose conj[rd, perm] -> cj[perm, (k, rd)] for batches 2,3
            for bl in (2, 3):
                psT = ps.tile([128, 192], B16, tag="ppt")
                for k in range(8):
                    nc.tensor.transpose(psT[:, k * 24:(k + 1) * 24],
                                        conj_sb[bl][:, k * 128:(k + 1) * 128],
                                        ident24[:])
                nc.scalar.activation(cj[:, bl * 192:(bl + 1) * 192], psT[:],
                                     AF.Copy)

            # batched tail: conj * PFOK, probsum over d, pd assembly
            nc.vector.tensor_tensor(cj[:], cj[:], PFOK[:], op=MUL)
            gA = wk.tile([128, 768], B16)
            nc.vector.tensor_scalar(gA[:], cj[:], -1.0, 1.0, op0=MUL, op1=ADD)
            d1 = wk.tile([128, 384], B16)
            gv = gA[:].rearrange("p (g dd) -> p g dd", dd=8)
            nc.vector.tensor_tensor(d1[:].rearrange("p (g dd) -> p g dd", dd=4),
                                    gv[:, :, 0:4], gv[:, :, 4:8], op=MUL)
            d2 = wk.tile([128, 192], B16)
            d1v = d1[:].rearrange("p (g dd) -> p g dd", dd=4)
            nc.vector.tensor_tensor(d2[:].rearrange("p (g dd) -> p g dd", dd=2),
                                    d1v[:, :, 0:2], d1v[:, :, 2:4], op=MUL)
            pdA2 = wk.tile([128, 96], B16)
            d2v = d2[:].rearrange("p (g dd) -> p g dd", dd=2)
            nc.vector.tensor_tensor(pdA2[:].unsqueeze(2),
                                    d2v[:, :, 0:1], d2v[:, :, 1:2], op=MUL)
            # relayout (b, t, r) -> (r, b, t) while casting to fp32
            pdF = wk.tile([128, 96], B16)
            nc.vector.tensor_copy(
                pdF[:].rearrange("p (r b t) -> p r b t", r=3, b=4),
                pdA2[:].rearrange("p (b t r) -> p b t r", b=4, t=8)
                    .transpose([0, 3, 1, 2]))

            # ---------- phase F: merges (all on-chip) ----------
            # binary last channel (row layout: [128, NT]); pd r=2 block
            tb = wk.tile([128, NT], F32)
            nc.vector.tensor_tensor(tb[:], obC[:], pdF[:, 64:96], op=MUL)
            nc.vector.tensor_scalar(tb[:], tb[:], -1.0, 1.0, op0=MUL, op1=ADD)
            nc.sync.dma_start(out_binm[:], tb[:])

            # transpose r=0 / r=1 pd blocks to [32 rows=(b,i8), 128=(i4,j)]
            # on the PE (identity matmul) instead of 8 DVE stream transposes
            r1Tp = ps.tile([32, 128], B16, tag="ppt")
            nc.tensor.transpose(r1Tp[:], pdF[:, 32:64], ident128[:])
            r0Tp = ps.tile([32, 128], B16, tag="ppt")
            nc.tensor.transpose(r0Tp[:], pdF[:, 0:32], ident128[:])

            # unary: product over j within each (b, i8, i4)
            pdu_t = wk.tile([32, 4], F32)
            nc.vector.tensor_reduce(
                pdu_t[:], r1Tp[:].rearrange("p (i4 j) -> p i4 j", i4=4),
                axis=AX, op=MUL)
            tu = wk.tile([32, 4], F32)
            nc.vector.tensor_tensor(tu[:], ouC[:], pdu_t[:], op=MUL)
            nc.vector.tensor_scalar(tu[:], tu[:], -1.0, 1.0, op0=MUL, op1=ADD)
            nc.sync.dma_start(out_unm[:], tu[:])

            # nullary: product over all (i, j) per b
            red0 = wk.tile([32, 1], F32)
            nc.vector.tensor_reduce(red0[:], r0Tp[:], axis=AX, op=MUL)
            # fold the remaining 32 partition values (b, i8) -> per-b products
            q = wk.tile([32, 32], F32)
            nc.vector.memset(q[:], 1.0)
            nc.vector.tensor_copy(q[:, 0:1], red0[:])
            qT = wk.tile([32, 32], F32)
            nc.vector.transpose(qT[:], q[:])
            pdn_t = wk.tile([1, 4], F32)
            nc.vector.tensor_reduce(
                pdn_t[:], qT[0:1, :].rearrange("p (b i8) -> p b i8", b=4),
                axis=AX, op=MUL)
            tn = wk.tile([1, 4], F32)
            nc.vector.tensor_tensor(tn[:], onC[:], pdn_t[:], op=MUL)
            nc.vector.tensor_scalar(tn[:], tn[:], -1.0, 1.0, op0=MUL, op1=ADD)
            nc.sync.dma_start(out_nullm[:], tn[:])
            _es2.close()

    nc.compile()
    return nc


def _host_prep(nullary_preds, unary_preds, binary_preds, and_kernel, or_kernel):
    """Build per-core input maps (sharding + layout prep only)."""
    null_ = np.asarray(nullary_preds, np.float32)
    un = np.asarray(unary_preds, np.float32)
    bi = np.asarray(binary_preds, np.float32)
    ak = np.asarray(and_kernel, np.float32)
    ok = np.asarray(or_kernel, np.float32)

    I, J = np.meshgrid(np.arange(N), np.arange(N), indexing="ij")
    off = I != J
    Jm = J - (J > I)
    Im = I - (I > J)

    binP = np.zeros((B, N, N, P2), np.float32)
    binP[:, off] = bi[:, I[off], Jm[off]]
    binT = np.zeros((B, N, N, P2), np.float32)
    binT[:, off] = bi[:, J[off], Im[off]]
    binPT = np.concatenate([binP, binT], axis=-1)          # [B,32,32,32]

    # row-tile layout: x_all[core][p, k=(b,t), c] = binPT[4c+b, t*128+p, c]
    xg = binPT.reshape(NCORE, BL, 8, 128, 32)
    x_all = np.ascontiguousarray(xg.transpose(0, 3, 1, 2, 4)
                                 ).reshape(NCORE, 128, NT * 32).astype(BF)
    # channel-major replicas of local batches 2,3 for the ACT-ln path:
    # xr[core][c + 32*r4, (bl-2)*1024 + i*32 + j] = binPT[core*4+bl, i, j, c]
    xrg = binPT.reshape(NCORE, BL, 1024, 32)[:, 2:4]        # [8, 2, 1024, 32]
    xrg = xrg.transpose(0, 3, 1, 2).reshape(NCORE, 1, 32, 2048)
    xr = np.broadcast_to(xrg, (NCORE, 4, 32, 2048)).reshape(
        NCORE, 128, 2048).astype(BF)
    olds_bin = np.ascontiguousarray(
        binP[..., 15].reshape(NCORE, BL, 8, 128).transpose(0, 3, 1, 2)
    ).reshape(NCORE, 128, NT).astype(np.float32)

    # unary pass rows (b, i): [u | u | n]
    xun = np.concatenate(
        [un, un, np.broadcast_to(null_[:, None, :], (B, N, P0))], axis=-1)
    xu = xun.reshape(NCORE, 128, 80).astype(BF)
    # rows (b, i8), cols i4 : out_unm[q=(b*8+i//4), i%4]
    olds_un = un[..., 31].reshape(NCORE, 4, 8, 4).reshape(NCORE, 32, 4).astype(np.float32)
    olds_null = null_[:, 15].reshape(NCORE, 1, 4).astype(np.float32)

    akT = np.ascontiguousarray(ak.transpose(2, 0, 1, 3)).reshape(112, 72)
    ork = ok.reshape(1, 24).astype(np.float32)

    p = np.arange(128)
    t = np.arange(8)
    selT = (np.arange(32)[:, None, None] == (t[None, :, None] * 4 + p[None, None, :] // 32))
    selJ = (np.arange(32)[:, None] == (p[None, :] % 32))
    selcat = np.concatenate([selT.reshape(32, 1024), selJ], axis=1).astype(BF)
    maskc = ((p[:, None] % 32) != (t[None, :] * 4 + p[:, None] // 32)
             ).astype(np.float32)

    in_maps = []
    for c in range(NCORE):
        in_maps.append({
            "x_all": x_all[c],
            "xr": np.ascontiguousarray(xr[c]),
            "xu": xu[c],
            "akt": akT,
            "ork": ork,
            "selcat": selcat,
            "maskc": maskc,
            "olds_bin": olds_bin[c],
            "olds_un": olds_un[c],
            "olds_null": olds_null[c],
        })
    return in_maps


def _assemble(results, nullary_preds, unary_preds, binary_preds):
    null_ = np.asarray(nullary_preds, np.float32).copy()
    un = np.asarray(unary_preds, np.float32).copy()
    bi = np.asarray(binary_preds, np.float32).copy()

    I, J = np.meshgrid(np.arange(N), np.arange(N), indexing="ij")
    off = I != J
    Jm = J - (J > I)

    for c in range(NCORE):
        r = results[c]
        # out_binm [128, NT=(b,t)] -> rows[b, t*128+p]
        ob = r["out_binm"].reshape(128, BL, 8).transpose(1, 2, 0).reshape(BL, N, N)
        for bl in range(BL):
            b = c * BL + bl
            bi[b, I[off], Jm[off], 15] = ob[bl][off]
        un[c * BL:(c + 1) * BL, :, 31] = r["out_unm"].reshape(BL, 8, 4).reshape(BL, N)
        null_[c * BL:(c + 1) * BL, 15] = r["out_nullm"].reshape(BL)

    return np.concatenate(
        [null_, un.reshape(B, -1), bi.reshape(B, -1)], axis=-1)


def kernel(nullary_preds, unary_preds, binary_preds, and_kernel, or_kernel):
    from concourse.bass_utils import run_bass_kernel_spmd

    if "nc" not in _CACHE:
        _CACHE["nc"] = _build()
    nc = _CACHE["nc"]

    in_maps = _host_prep(nullary_preds, unary_preds, binary_preds,
                         and_kernel, or_kernel)
    res = run_bass_kernel_spmd(nc, in_maps, list(range(NCORE)))
    return _assemble(res.results, nullary_preds, unary_preds, binary_preds)


if __name__ == "__main__":
    import reference as ref
    ins = {k: np.asarray(v) for k, v in ref.setup_inputs().items()}
    out = kernel(**ins)
    print("kernel out:", out.shape, out.dtype)

